# revision 1
# baseline (speedup 1.0000x reference)
"""Trainium2 Bass kernel for nn_CrossAttention_17033840296537.

Full-input contract: kernel(**inputs) takes the unsharded tensors as in
reference.setup_inputs() and returns the full [8, 2048, 512] output.

Sharding: data-parallel over batch B=8 across the 8 NeuronCores (one
batch element per core). Weights are replicated.

Per-core algorithm (all matmuls in float32r, N=512 free dims):
  prologue (on-chip PE transposes, scratch in DRAM):
    qk_w^T  [512c, 512hd]   (SBUF, used for projections)
    fc_w^T  [4096hd, 512o]  -> DRAM scratch
    v_w^T   [512c, 4096hd]  -> DRAM scratch
    q^T, k^T (chunked)      -> projected immediately:
    qh^T = qk_w @ q^T [512hd, 2048q] -> DRAM scratch (same for kh^T)
  main loop over heads h, query chunks j (512 wide):
    scores^T[s, q] = kh^T[h].T-slices @ qh^T[h]   (K=64, direct in [s,q]
        layout so softmax needs no transpose)
    P^T = exp(scores^T * 0.125 + (-1e4 * mask)[s])  one ACT op, fused
        temperature + mask; no max-subtraction needed (|scores/8| <~ 8).
    T1[c, q]   = sum_s v[s, c] P^T[s, q]          (lhsT = v as stored!)
    r[q]       = sum_s P^T[s, q]                  (ones-vector matmul)
    out_h^T    = wv[h]^T-slices @ T1  then * (1/r) broadcast
    fc partial = out_h^T-slices @ fc_w^T[h]  accumulated over h in SBUF
  epilogue: += idt, LayerNorm, -> out.
"""

import numpy as np

import concourse.bass as bass
import concourse.tile as tile
from concourse import mybir
from concourse.bass import ds
from concourse.masks import make_identity

F32 = mybir.dt.float32
FR = mybir.dt.float32r
I32 = mybir.dt.int32
AF = mybir.ActivationFunctionType

B = 8
NQ = NS = 2048
DIM = 512          # input channel dim (DIM_K == DIM_V == 512)
N_HEAD = 8
D_K = 64
D_V = 512
HD = N_HEAD * D_V  # 4096 concat dim
P = 128


def fr(ap):
    return ap.bitcast(FR)


def _emit(tc: tile.TileContext, io: dict):
    nc = tc.nc
    q, k, v, mask, idt = io["q"], io["k"], io["v"], io["mask"], io["idt"]
    qk_w, v_w, fc_w = io["qk_w"], io["v_w"], io["fc_w"]
    fc_b, ln_g, ln_b = io["fc_b"], io["ln_g"], io["ln_b"]
    out = io["out"]
    qhT_d, khT_d = io["qhT_d"], io["khT_d"]

    cpool_cm = tc.tile_pool(name="cpool", bufs=1)
    vpool_cm = tc.tile_pool(name="vpool", bufs=1)
    cpool = cpool_cm.__enter__()
    vpool = vpool_cm.__enter__()

    # ---- constants ----
    ident = cpool.tile([P, P], F32)
    make_identity(nc, ident)
    ones_f = cpool.tile([P, P], F32)
    nc.vector.memset(ones_f, 1.0)
    ones32 = cpool.tile([P, 32], FR)
    nc.vector.tensor_copy(out=ones32, in_=ones_f[:, 0:32])
    ones_row = cpool.tile([1, P], FR)
    nc.vector.tensor_copy(out=ones_row, in_=ones_f[0:1, :])
    eps_t = cpool.tile([P, 1], F32)
    nc.vector.memset(eps_t, 1e-5)

    mask_i = cpool.tile([P, 16], I32)
    nc.sync.dma_start(out=mask_i, in_=mask.rearrange("(a p) -> p a", p=P))
    mask_b = cpool.tile([P, 16], F32)
    nc.vector.tensor_copy(out=mask_b, in_=mask_i)  # int32 -> f32 cast
    nc.scalar.mul(mask_b, mask_b, -10000.0)

    def bcast_row(name, src):  # [512] dram -> [128, 512] sbuf (rows identical)
        bc = cpool.tile([P, D_V], F32, name=name + "_bc")
        src_b = bass.AP(tensor=src.tensor, offset=src.offset,
                        ap=[[0, P]] + list(src.ap))
        nc.sync.dma_start(out=bc, in_=src_b)
        return bc

    fcb_bc = bcast_row("fcb", fc_b)
    lng_bc = bcast_row("lng", ln_g)
    lnb_bc = bcast_row("lnb", ln_b)

    # ---- resident: v tiles and fc accumulator ----
    vt = []
    for sb in range(16):
        vstg = vpool.tile([P, DIM], F32, name=f"vstg{sb}", tag="vstg", bufs=3)
        nc.gpsimd.dma_start(out=vstg, in_=v[ds(sb * P, P), :])
        vts = vpool.tile([P, DIM], FR, name=f"v{sb}", tag=f"v{sb}")
        nc.vector.tensor_copy(out=vts, in_=vstg)
        vt.append(vts)
    facc = [vpool.tile([P, D_V], F32, name=f"facc{i}", tag=f"facc{i}")
            for i in range(16)]

    # ================= prologue =================
    with (
        tc.tile_pool(name="ld", bufs=2) as ld,
        tc.tile_pool(name="ppsum", bufs=1, space="PSUM") as ppsum,
        tc.tile_pool(name="wpool", bufs=1) as wpool,
    ):
        # qk_w^T [c, hd] stays in SBUF for the projections below
        qkwT = [wpool.tile([P, 512], FR, name=f"qkwT{cb}", tag=f"qkwT{cb}")
                for cb in range(4)]
        wrows = []
        for rb in range(4):
            wrow = ld.tile([P, 512], F32, tag="wrow", bufs=4)
            nc.sync.dma_start(out=wrow, in_=qk_w[ds(rb * P, P), :])
            wrows.append(wrow)
        for cb in range(4):
            tp4 = ppsum.tile([P, 512], F32, tag="tp4", bufs=3)
            for rb in range(4):
                nc.tensor.transpose(tp4[:, ds(rb * P, P)],
                                    wrows[rb][:, ds(cb * P, P)], ident)
            nc.vector.tensor_copy(out=qkwT[cb], in_=tp4)

        # q, k: transpose chunks + project through qk_w^T -> qh^T/kh^T scratch
        for src, dst in ((q, qhT_d), (k, khT_d)):
            for j2 in range(4):  # chunks of 512 sequence rows
                qts = []
                for qb in range(4):
                    qt = ld.tile([P, DIM], F32, tag="qld", bufs=8)
                    nc.gpsimd.dma_start(out=qt, in_=src[ds((j2 * 4 + qb) * P, P), :])
                    qts.append(qt)
                qTc = [ld.tile([P, 512], FR, name=f"qTc{cb}", tag=f"qTc{cb}")
                       for cb in range(4)]
                for cb in range(4):
                    tp4 = ppsum.tile([P, 512], F32, tag="tp4", bufs=3)
                    for qb in range(4):
                        nc.tensor.transpose(tp4[:, ds(qb * P, P)],
                                            qts[qb][:, ds(cb * P, P)], ident)
                    nc.vector.tensor_copy(out=qTc[cb], in_=tp4)
                for mb in range(4):
                    pr = ppsum.tile([P, 512], F32, tag="pr", bufs=2)
                    for cb in range(4):
                        nc.tensor.matmul(pr, lhsT=qkwT[cb][:, ds(mb * P, P)],
                                         rhs=qTc[cb],
                                         start=(cb == 0), stop=(cb == 3))
                    prs = ld.tile([P, 512], FR, tag="prs", bufs=2)
                    nc.vector.tensor_copy(out=prs, in_=pr)
                    nc.sync.dma_start(out=dst[ds(mb * P, P), ds(j2 * 512, 512)],
                                      in_=prs)

    # ================= main =================
    with (
        tc.tile_pool(name="mpsum", bufs=1, space="PSUM") as mpsum,
        tc.tile_pool(name="hpool", bufs=1) as hpool,
        tc.tile_pool(name="spool", bufs=2) as spool,
    ):
        for h in range(N_HEAD):
            # qh^T / kh^T for this head, duplicated into both partition
            # halves so paired score matmuls can row-tile the PE array.
            qh2 = hpool.tile([P, NQ], FR, tag="qh", bufs=2)
            nc.gpsimd.dma_start(out=qh2[0:D_K, :], in_=qhT_d[ds(h * D_K, D_K), :])
            nc.gpsimd.dma_start(out=qh2[D_K:P, :], in_=qhT_d[ds(h * D_K, D_K), :])
            kh2 = hpool.tile([P, NS], FR, tag="kh", bufs=2)
            nc.gpsimd.dma_start(out=kh2[0:D_K, :], in_=khT_d[ds(h * D_K, D_K), :])
            nc.gpsimd.dma_start(out=kh2[D_K:P, :], in_=khT_d[ds(h * D_K, D_K), :])

            # transpose this head's slices of v_w and fc_w on the fly
            wvT = [hpool.tile([P, 512], FR, name=f"wvT{cb}", tag=f"wv{cb}")
                   for cb in range(4)]
            vwr = []
            for i in range(4):
                vw_raw = hpool.tile([P, 512], F32, tag=f"raw{i}")
                nc.gpsimd.dma_start(out=vw_raw, in_=v_w[ds((h * 4 + i) * P, P), :])
                vwr.append(vw_raw)
            for cb in range(4):
                tp4 = mpsum.tile([P, 512], F32, tag="sc", bufs=2)
                for i in range(4):
                    nc.tensor.transpose(tp4[:, ds(i * P, P)],
                                        vwr[i][:, ds(cb * P, P)], ident)
                nc.vector.tensor_copy(out=wvT[cb], in_=tp4)
            fwT = [hpool.tile([P, 512], FR, name=f"fwT{db}", tag=f"fw{db}")
                   for db in range(4)]
            fwr = []
            for rb in range(4):
                fw_raw = hpool.tile([P, 512], F32, tag=f"raw{rb}")
                nc.gpsimd.dma_start(out=fw_raw,
                                  in_=fc_w[ds(rb * P, P), ds(h * 512, 512)])
                fwr.append(fw_raw)
            for db in range(4):
                tp4 = mpsum.tile([P, 512], F32, tag="sc", bufs=2)
                for rb in range(4):
                    nc.tensor.transpose(tp4[:, ds(rb * P, P)],
                                        fwr[rb][:, ds(db * P, P)], ident)
                nc.vector.tensor_copy(out=fwT[db], in_=tp4)

            for j in range(4):  # query chunks of 512
                t1 = mpsum.tile([P, 4 * 512], F32, tag="t1", bufs=1)
                r2a = mpsum.tile([32, 512], F32, tag="oo", bufs=2)

                def emit_sc_pair(si):
                    # paired score matmuls row-tiled into PE halves
                    sba, sbb = 2 * si, 2 * si + 1
                    sc_a = mpsum.tile([P, 512], F32, tag="sc", bufs=2,
                                      name=f"sc_a{si}")
                    nc.tensor.matmul(sc_a, lhsT=kh2[0:D_K, ds(sba * P, P)],
                                     rhs=qh2[0:D_K, ds(j * 512, 512)],
                                     start=True, stop=True,
                                     tile_position=(0, 0))
                    sc_b = mpsum.tile([P, 512], F32, tag="sc", bufs=2,
                                      name=f"sc_b{si}")
                    nc.tensor.matmul(sc_b, lhsT=kh2[D_K:P, ds(sbb * P, P)],
                                     rhs=qh2[D_K:P, ds(j * 512, 512)],
                                     start=True, stop=True,
                                     tile_position=(D_K, 0))
                    return sc_a, sc_b

                def emit_half(sb, sc, rrow):
                    pt = spool.tile([P, 512], FR, tag="pt", bufs=6)
                    nc.scalar.activation(pt, sc, AF.Exp,
                                         bias=mask_b[:, ds(sb, 1)],
                                         scale=0.125)
                    for cb in range(4):
                        nc.tensor.matmul(t1[:, ds(cb * 512, 512)],
                                         lhsT=vt[sb][:, ds(cb * P, P)],
                                         rhs=pt,
                                         start=(sb == 0), stop=(sb == 15))
                    # softmax denominator partials (one accumulation group)
                    nc.tensor.matmul(r2a[0:32, :], lhsT=ones32, rhs=pt,
                                     start=(sb == 0), stop=(sb == 15))

                pair = emit_sc_pair(0)
                for si in range(8):
                    sc_a, sc_b = pair
                    emit_half(2 * si, sc_a, 0)
                    if si < 7:
                        pair = emit_sc_pair(si + 1)
                    emit_half(2 * si + 1, sc_b, 32)

                rs = spool.tile([1, 512], FR, tag="rs")
                with nc.allow_low_precision(reason="f32r rounding of 1/r"):
                    nc.vector.reciprocal(rs, r2a[0:1, :])
                # broadcast 1/r across partitions via K=1 ones-matmul, then
                # move to SBUF right away so the PSUM slot frees quickly
                rb_t = mpsum.tile([P, 512], F32, tag="oo", bufs=2)
                nc.tensor.matmul(rb_t, lhsT=ones_row, rhs=rs,
                                 start=True, stop=True)
                rbs = spool.tile([P, 512], F32, tag="rbs", bufs=2)
                nc.vector.tensor_copy(out=rbs, in_=rb_t)
                # t1 PSUM -> SBUF on the scalar engine (idle here), chunked so
                # stage-2 matmuls can start on chunk 0 early
                t1s = spool.tile([P, 4 * 512], FR, tag="t1s", bufs=2)
                for cb in range(4):
                    nc.scalar.copy(out=t1s[:, ds(cb * 512, 512)],
                                   in_=t1[:, ds(cb * 512, 512)])

                oTs = []
                for db in range(4):
                    oo = mpsum.tile([P, 512], F32, tag="oo", bufs=2)
                    for cb in range(4):
                        nc.tensor.matmul(oo, lhsT=wvT[cb][:, ds(db * P, P)],
                                         rhs=t1s[:, ds(cb * 512, 512)],
                                         start=(cb == 0), stop=(cb == 3))
                    oT = spool.tile([P, 512], FR, name=f"oT{db}", tag=f"oT{db}")
                    nc.vector.tensor_mul(oT, oo, rbs)
                    oTs.append(oT)

                for qb in range(4):
                    fp = mpsum.tile([P, 512], F32, tag="oo", bufs=2)
                    for db in range(4):
                        nc.tensor.matmul(fp, lhsT=oTs[db][:, ds(qb * P, P)],
                                         rhs=fwT[db],
                                         start=(db == 0), stop=(db == 3))
                    i16 = j * 4 + qb
                    if h == 0:
                        nc.vector.tensor_add(facc[i16], fp, fcb_bc)
                    else:
                        nc.vector.tensor_add(facc[i16], fp, facc[i16])

        # ---- epilogue: residual + LayerNorm ----
        for i16 in range(16):
            it = spool.tile([P, D_V], F32, tag="it")
            nc.gpsimd.dma_start(out=it, in_=idt[ds(i16 * P, P), :])
            xt = spool.tile([P, D_V], F32, tag="xt")
            nc.vector.tensor_add(xt, facc[i16], it)
            st = spool.tile([P, 6], F32, tag="st")
            nc.vector.bn_stats(out=st, in_=xt)
            mv = spool.tile([P, 2], F32, tag="mv")
            nc.vector.bn_aggr(out=mv, in_=st)
            sd = spool.tile([P, 1], F32, tag="sd")
            nc.scalar.activation(sd, mv[:, 1:2], AF.Sqrt, bias=eps_t)
            rstd = spool.tile([P, 1], F32, tag="rstd")
            nc.vector.reciprocal(rstd, sd)
            nc.vector.tensor_scalar(out=xt, in0=xt, scalar1=mv[:, 0:1],
                                    scalar2=rstd,
                                    op0=mybir.AluOpType.subtract,
                                    op1=mybir.AluOpType.mult)
            nc.vector.tensor_mul(xt, xt, lng_bc)
            nc.vector.tensor_add(xt, xt, lnb_bc)
            nc.sync.dma_start(out=out[ds(i16 * P, P), :], in_=xt)

    vpool_cm.__exit__(None, None, None)
    cpool_cm.__exit__(None, None, None)


def build_nc():
    from concourse import bacc
    nc = bacc.Bacc("TRN2", target_bir_lowering=False, debug=False)
    io = {}
    io["q"] = nc.dram_tensor("q", [NQ, DIM], F32, kind="ExternalInput").ap()
    io["k"] = nc.dram_tensor("k", [NS, DIM], F32, kind="ExternalInput").ap()
    io["v"] = nc.dram_tensor("v", [NS, DIM], F32, kind="ExternalInput").ap()
    io["mask"] = nc.dram_tensor("mask", [NS], I32, kind="ExternalInput").ap()
    io["idt"] = nc.dram_tensor("idt", [NQ, D_V], F32, kind="ExternalInput").ap()
    io["qk_w"] = nc.dram_tensor("qk_w", [512, DIM], F32, kind="ExternalInput").ap()
    io["v_w"] = nc.dram_tensor("v_w", [HD, DIM], F32, kind="ExternalInput").ap()
    io["fc_w"] = nc.dram_tensor("fc_w", [D_V, HD], F32, kind="ExternalInput").ap()
    io["fc_b"] = nc.dram_tensor("fc_b", [D_V], F32, kind="ExternalInput").ap()
    io["ln_g"] = nc.dram_tensor("ln_g", [D_V], F32, kind="ExternalInput").ap()
    io["ln_b"] = nc.dram_tensor("ln_b", [D_V], F32, kind="ExternalInput").ap()
    io["out"] = nc.dram_tensor("out", [NQ, D_V], F32, kind="ExternalOutput").ap()
    io["qhT_d"] = nc.dram_tensor("qhT_d", [512, NQ], FR).ap()
    io["khT_d"] = nc.dram_tensor("khT_d", [512, NS], FR).ap()

    with tile.TileContext(nc) as tc:
        _emit(tc, io)
    nc.compile()
    return nc


_NC = None


def get_nc():
    global _NC
    if _NC is None:
        _NC = build_nc()
    return _NC


def make_in_maps(q, k, v, s_valid_mask, idt, qk_w, v_w, fc_w, fc_b, ln_g, ln_b):
    in_maps = []
    for b in range(B):
        in_maps.append({
            "q": np.ascontiguousarray(q[b], dtype=np.float32),
            "k": np.ascontiguousarray(k[b], dtype=np.float32),
            "v": np.ascontiguousarray(v[b], dtype=np.float32),
            "mask": np.ascontiguousarray(s_valid_mask[b], dtype=np.int32),
            "idt": np.ascontiguousarray(idt[b], dtype=np.float32),
            "qk_w": np.ascontiguousarray(qk_w, dtype=np.float32),
            "v_w": np.ascontiguousarray(v_w, dtype=np.float32),
            "fc_w": np.ascontiguousarray(fc_w, dtype=np.float32),
            "fc_b": np.ascontiguousarray(fc_b, dtype=np.float32),
            "ln_g": np.ascontiguousarray(ln_g, dtype=np.float32),
            "ln_b": np.ascontiguousarray(ln_b, dtype=np.float32),
        })
    return in_maps


def kernel(q, k, v, s_valid_mask, idt, qk_w, v_w, fc_w, fc_b, ln_g, ln_b,
           **run_kwargs):
    from concourse.bass_utils import run_bass_kernel_spmd

    nc = get_nc()
    in_maps = make_in_maps(q, k, v, s_valid_mask, idt,
                           qk_w, v_w, fc_w, fc_b, ln_g, ln_b)
    res = run_bass_kernel_spmd(nc, in_maps, core_ids=list(range(B)),
                               **run_kwargs)
    out = np.stack([res.results[b]["out"] for b in range(B)], axis=0)
    kernel.last_results = res
    return out.astype(np.float32)

